# revision 5
# baseline (speedup 1.0000x reference)
"""AddShift segment-reduce kernel for Trainium2, SPMD across 8 NeuronCores.

Problem (hardcoded): x [8, 1792, 4096] f32; channel c = g*448 + co*7 + k
(g<4, co<64, k<7). For each of 3 index sets m, channel c is shifted along
the last axis by s_m(c) = REAL_PAD[shift_idx_m[c] % 7] in {+-15,+-10,+-5,0}
(zero fill) and channels sharing co are summed over (g, k):
  out_m[b, co, t] = sum_{g,k} x[b, c, t - s_m(c)] * in_range
Returns (x1, x2, x3), each [8, 64, 4096] f32.

Sharding: data-parallel over batch — core b computes batch b.

Device algorithm (3 TensorE stages, all bf16 operands, f32 accumulation):
  S1  per channel-block (4 co slots = 112 channels) and 512-col chunk:
      one matmul with a 0/1 selector stationary [112, 84] reduces channels
      into 84 bucket rows (v shift-class x m set x co_l) in PSUM;
      PSUM is drained to SBUF as bf16 (P).
  T   PE-transposes P in 128-col chunks into PT [t, bucket], reordering
      buckets v-major: col = v*192 + m*64 + co.
  S2  banded matmuls: for dst t-chunk j, out[t', (m,co)] += B.T @ PT_v
      where B is a shifted-identity band [128, 128]; corner bands pull the
      <=15-row contributions that cross chunk boundaries from PT[j-1]/PT[j+1].
      Boundary zero-fill falls out of omitted corners / zero band columns.
Output is written transposed (outT [4096, 192]); the host restores layout.

x is shipped as bf16 (host cast) — halves HBM traffic; selector/band
weights are exact 0/1 in bf16. Overall rel. error ~3e-3.
"""
from contextlib import ExitStack

import numpy as np
import ml_dtypes

import concourse.bass as bass
import concourse.mybir as mybir
import concourse.tile as tile
from concourse import bacc
from concourse.bass_utils import run_bass_kernel_spmd

# Problem constants
SMALL_K = 5
C_OUT, GROUP_IN = 64, 4
NK = 7
C_IN = 1792
B, L = 8, 4096
REAL_PAD = [int(((NK - 1) / 2.0 - i) * SMALL_K) for i in range(NK)]  # [15,10,5,0,-5,-10,-15]
N_SETS = 3
CO_PER_BLK = 4
CH_PER_BLK = CO_PER_BLK * GROUP_IN * NK  # 112
N_BLOCKS = C_OUT // CO_PER_BLK           # 16
NCO = C_OUT
W = N_SETS * NCO                          # 192
TCH = 128
CHUNK = 512

F32 = mybir.dt.float32
BF16 = mybir.dt.bfloat16

CORNER_OF = {0: 7, 1: 8, 2: 9, 4: 10, 5: 11, 6: 12}
V_OF_BAND = {0: 0, 1: 1, 2: 2, 3: 3, 4: 4, 5: 5, 6: 6,
             7: 0, 8: 1, 9: 2, 10: 4, 11: 5, 12: 6}

_compiled = None


def _permute_rows():
    idx = np.empty(CH_PER_BLK * N_BLOCKS, np.int64)
    for i in range(N_BLOCKS):
        for p in range(CH_PER_BLK):
            g, r = divmod(p, 28)
            co_l, k = divmod(r, NK)
            idx[i * CH_PER_BLK + p] = g * (C_OUT * NK) + (CO_PER_BLK * i + co_l) * NK + k
    return idx


def _build_weights(shift_idxs):
    """S1 selector [112, N_BLOCKS*84]; block i cols (v*12 + m*4 + co_l)."""
    shifts = [np.asarray([REAL_PAD[int(ix) % NK] for ix in s], np.int32)
              for s in shift_idxs]
    Wm = np.zeros((CH_PER_BLK, N_BLOCKS, NK, N_SETS * CO_PER_BLK), np.float32)
    for i in range(N_BLOCKS):
        for p in range(CH_PER_BLK):
            g, r = divmod(p, 28)
            co_l, k = divmod(r, NK)
            c = g * (C_OUT * NK) + (CO_PER_BLK * i + co_l) * NK + k
            for m in range(N_SETS):
                v = REAL_PAD.index(int(shifts[m][c]))
                Wm[p, i, v, m * CO_PER_BLK + co_l] = 1.0
    return np.ascontiguousarray(Wm.reshape(CH_PER_BLK, N_BLOCKS * NK * 12))


def _build_bands():
    """[128, 13*128]: 7 main shifted-identity bands + 6 corner bands."""
    bands = np.zeros((13, 128, 128), np.float32)
    for vi, s in enumerate(REAL_PAD):
        for u in range(128):
            t = u + s
            if 0 <= t < 128:
                bands[vi, u, t] = 1.0
    ci = 7
    for vi, s in enumerate(REAL_PAD):
        if s == 0:
            continue
        for u in range(128):
            t = u + s - 128 if s > 0 else u + s + 128
            if 0 <= t < 128:
                bands[ci, u, t] = 1.0
        ci += 1
    return np.ascontiguousarray(np.transpose(bands, (1, 0, 2)).reshape(128, 13 * 128))


def _build_kernel():
    n_chunks = L // CHUNK
    n_t = L // TCH
    tpc = CHUNK // TCH
    PW = 1024  # rolling P window (columns)

    nc = bacc.Bacc(None, target_bir_lowering=False, debug=False)
    x = nc.declare_dram_parameter("x", [C_IN, L], BF16, isOutput=False)
    w = nc.declare_dram_parameter("w", [CH_PER_BLK, N_BLOCKS * 84], BF16, isOutput=False)
    ident = nc.declare_dram_parameter("ident", [128, 128], BF16, isOutput=False)
    bands = nc.declare_dram_parameter("bands", [128, 13 * 128], BF16, isOutput=False)
    outT = nc.declare_dram_parameter("outT", [L, W], F32, isOutput=True)

    with tile.TileContext(nc) as tc, ExitStack() as ctx:
        const = ctx.enter_context(tc.tile_pool(name="const", bufs=1))
        xpool = ctx.enter_context(tc.tile_pool(name="x", bufs=N_BLOCKS + 8))
        ppool = ctx.enter_context(tc.tile_pool(name="P", bufs=1))
        ptpool = ctx.enter_context(tc.tile_pool(name="PT", bufs=4))
        otpool = ctx.enter_context(tc.tile_pool(name="oT", bufs=4))
        psA = ctx.enter_context(tc.tile_pool(name="psA", bufs=4, space=bass.MemorySpace.PSUM))
        psT = ctx.enter_context(tc.tile_pool(name="psT", bufs=2, space=bass.MemorySpace.PSUM))
        psS2 = ctx.enter_context(tc.tile_pool(name="psS2", bufs=2, space=bass.MemorySpace.PSUM))

        w_sb = const.tile([CH_PER_BLK, N_BLOCKS * 84], BF16)
        nc.sync.dma_start(w_sb[:], w[:])
        id_sb = const.tile([128, 128], BF16)
        nc.sync.dma_start(id_sb[:], ident[:])
        bd_sb = const.tile([128, 13 * 128], BF16)
        nc.sync.dma_start(bd_sb[:], bands[:])

        P_sb = [ppool.tile([84, PW], BF16, name=f"Psb{i}", tag=f"P{i}")
                for i in range(N_BLOCKS)]
        n_pairs = n_t // 2
        PTP = [None] * n_pairs  # pair tile cols: v*384 + m*128 + h*64 + co

        def emit_s2_pair(pi):
            """S2 for dst chunks (2pi, 2pi+1). Main bands cover both chunks
            in one [128, 2W] matmul; corners pull cross-chunk slices."""
            ja = 2 * pi

            def hview(tile_ap, vi, h):
                """[p, m, co] view of one (v, h) slice; cols v*384+m*128+h*64+co."""
                return tile_ap[:].rearrange("p (v m h co) -> p v m h co",
                                            v=NK, m=N_SETS, h=2)[:, vi, :, h, :]

            ps = psS2.tile([128, 2 * W], F32)
            ps_r = ps[:].rearrange("p (m h co) -> p m h co", m=N_SETS, h=2)
            ops = []  # (band, moving_ap, out_ap)
            for vi in range(NK):
                ops.append((vi, PTP[pi][:, vi * 2 * W:(vi + 1) * 2 * W], ps[:]))
            for vi in (0, 1, 2):  # s>0: dst h <- src chunk j-1
                if pi > 0:
                    ops.append((CORNER_OF[vi], hview(PTP[pi - 1], vi, 1),
                                ps_r[:, :, 0, :]))
                ops.append((CORNER_OF[vi], hview(PTP[pi], vi, 0), ps_r[:, :, 1, :]))
            for vi in (4, 5, 6):  # s<0: dst h <- src chunk j+1
                ops.append((CORNER_OF[vi], hview(PTP[pi], vi, 1), ps_r[:, :, 0, :]))
                if pi + 1 < n_pairs:
                    ops.append((CORNER_OF[vi], hview(PTP[pi + 1], vi, 0),
                                ps_r[:, :, 1, :]))
            for k, (b, mv, out_ap) in enumerate(ops):
                nc.tensor.matmul(out_ap, bd_sb[:, TCH * b:TCH * (b + 1)], mv,
                                 start=(k == 0), stop=(k == len(ops) - 1))
            ot = otpool.tile([128, 2 * W], F32)
            nc.scalar.copy(ot[:], ps[:])
            ot_r = ot[:].rearrange("p (m h co) -> p m h co", m=N_SETS, h=2)
            for h in range(2):
                nc.scalar.dma_start(outT[TCH * (ja + h):TCH * (ja + h + 1), :],
                                    ot_r[:, :, h, :])

        XW = 2 * CHUNK
        xslabs = []
        for jc in range(n_chunks):
            if jc % 2 == 0:
                xslabs = []
                for i in range(N_BLOCKS):
                    xt = xpool.tile([CH_PER_BLK, XW], BF16)
                    nc.sync.dma_start(xt[:], x[CH_PER_BLK * i:CH_PER_BLK * (i + 1),
                                               CHUNK * jc:CHUNK * jc + XW])
                    xslabs.append(xt)
            xts = [xs[:, (jc % 2) * CHUNK:(jc % 2 + 1) * CHUNK] for xs in xslabs]
            for i in range(N_BLOCKS):
                ps = psA.tile([84, CHUNK], F32)
                nc.tensor.matmul(ps[:], w_sb[:, 84 * i:84 * (i + 1)], xts[i],
                                 start=True, stop=True)
                pc = (CHUNK * jc) % PW
                if i % 2 == 0:
                    nc.vector.tensor_copy(P_sb[i][:, pc:pc + CHUNK], ps[:])
                else:
                    nc.scalar.copy(P_sb[i][:, pc:pc + CHUNK], ps[:])
            for jt in range(tpc * jc, tpc * (jc + 1)):
                pi, hh = divmod(jt, 2)
                if hh == 0:
                    PTP[pi] = ptpool.tile([128, NK * 2 * W], BF16, name=f"ptp{pi}", tag="ptp")
                pt = PTP[pi]
                for h0 in range(0, N_BLOCKS, 8):
                    hblks = list(range(h0, min(h0 + 8, N_BLOCKS)))
                    pst = psT.tile([128, 84 * 8], BF16)
                    for q, i in enumerate(hblks):
                        tcol = (TCH * jt) % PW
                        nc.tensor.transpose(pst[:, 84 * q:84 * (q + 1)],
                                            P_sb[i][:, tcol:tcol + TCH],
                                            id_sb[:84, :84])
                    nq = len(hblks)
                    csrc = pst[:, 0:84 * nq].rearrange(
                        "p (q v m c) -> p q v m c", q=nq, v=NK, m=N_SETS)
                    cdst = pt[:].rearrange(
                        "p (v m h co) -> p v m h co", v=NK, m=N_SETS, h=2)[
                        :, :, :, hh,
                        CO_PER_BLK * h0:CO_PER_BLK * (h0 + nq)].rearrange(
                        "p v m (q c) -> p q v m c", q=nq)
                    if h0 == 0:
                        nc.scalar.copy(cdst, csrc)
                    else:
                        nc.vector.tensor_copy(cdst, csrc)
                if jt >= 2 and jt % 2 == 0:
                    emit_s2_pair(jt // 2 - 1)
        emit_s2_pair(n_pairs - 1)

    nc.compile()
    return nc


def kernel(x, shift_idx_1, shift_idx_2, shift_idx_3):
    global _compiled
    x = np.asarray(x)
    sidx = [np.asarray(shift_idx_1), np.asarray(shift_idx_2), np.asarray(shift_idx_3)]

    if _compiled is None:
        _compiled = _build_kernel()
    nc = _compiled

    Wm = _build_weights(sidx).astype(ml_dtypes.bfloat16)
    bands = _build_bands().astype(ml_dtypes.bfloat16)
    ident = np.eye(128, dtype=np.float32).astype(ml_dtypes.bfloat16)
    ridx = _permute_rows()
    in_maps = []
    for b in range(B):
        xb = np.ascontiguousarray(np.asarray(x[b], np.float32)[ridx]).astype(ml_dtypes.bfloat16)
        in_maps.append({"x": xb, "w": Wm, "ident": ident, "bands": bands})

    res = run_bass_kernel_spmd(nc, in_maps, core_ids=list(range(8)), trace=False)

    outs = []
    for m in range(N_SETS):
        outs.append(np.stack(
            [np.asarray(res.results[b]["outT"][:, m * NCO:(m + 1) * NCO].T,
                        dtype=np.float32) for b in range(B)]))
    return tuple(outs)


# revision 7
# speedup vs baseline: 1.1363x; 1.1363x over previous
"""AddShift segment-reduce kernel for Trainium2, SPMD across 8 NeuronCores.

Problem (hardcoded): x [8, 1792, 4096] f32; channel c = g*448 + co*7 + k
(g<4, co<64, k<7). For each of 3 index sets m, channel c is shifted along
the last axis by s_m(c) = REAL_PAD[shift_idx_m[c] % 7] in {+-15,+-10,+-5,0}
(zero fill) and channels sharing co are summed over (g, k):
  out_m[b, co, t] = sum_{g,k} x[b, c, t - s_m(c)] * in_range
Returns (x1, x2, x3), each [8, 64, 4096] f32.

Sharding: data-parallel over batch — core b computes batch b.

Device algorithm (3 TensorE stages, all bf16 operands, f32 accumulation):
  S1  per channel-block (4 co slots = 112 channels) and 512-col chunk:
      one matmul with a 0/1 selector stationary [112, 84] reduces channels
      into 84 bucket rows (v shift-class x m set x co_l) in PSUM;
      PSUM is drained to SBUF as bf16 (P).
  T   PE-transposes P in 128-col chunks into PT [t, bucket], reordering
      buckets v-major: col = v*192 + m*64 + co.
  S2  banded matmuls: for dst t-chunk j, out[t', (m,co)] += B.T @ PT_v
      where B is a shifted-identity band [128, 128]; corner bands pull the
      <=15-row contributions that cross chunk boundaries from PT[j-1]/PT[j+1].
      Boundary zero-fill falls out of omitted corners / zero band columns.
Output is written transposed (outT [4096, 192]); the host restores layout.

x is shipped as bf16 (host cast) — halves HBM traffic; selector/band
weights are exact 0/1 in bf16. Overall rel. error ~3e-3.
"""
from contextlib import ExitStack

import numpy as np
import ml_dtypes

import concourse.bass as bass
import concourse.mybir as mybir
import concourse.tile as tile
from concourse import bacc
from concourse.bass_utils import run_bass_kernel_spmd

# Problem constants
SMALL_K = 5
C_OUT, GROUP_IN = 64, 4
NK = 7
C_IN = 1792
B, L = 8, 4096
REAL_PAD = [int(((NK - 1) / 2.0 - i) * SMALL_K) for i in range(NK)]  # [15,10,5,0,-5,-10,-15]
N_SETS = 3
CO_PER_BLK = 4
CH_PER_BLK = CO_PER_BLK * GROUP_IN * NK  # 112
N_BLOCKS = C_OUT // CO_PER_BLK           # 16
NCO = C_OUT
W = N_SETS * NCO                          # 192
TCH = 128
CHUNK = 512

F32 = mybir.dt.float32
BF16 = mybir.dt.bfloat16

CORNER_OF = {0: 7, 1: 8, 2: 9, 4: 10, 5: 11, 6: 12}
V_OF_BAND = {0: 0, 1: 1, 2: 2, 3: 3, 4: 4, 5: 5, 6: 6,
             7: 0, 8: 1, 9: 2, 10: 4, 11: 5, 12: 6}

_compiled = None


def _permute_rows():
    idx = np.empty(CH_PER_BLK * N_BLOCKS, np.int64)
    for i in range(N_BLOCKS):
        for p in range(CH_PER_BLK):
            g, r = divmod(p, 28)
            co_l, k = divmod(r, NK)
            idx[i * CH_PER_BLK + p] = g * (C_OUT * NK) + (CO_PER_BLK * i + co_l) * NK + k
    return idx


def _build_weights(shift_idxs):
    """S1 selector [112, N_BLOCKS*84]; block i cols (v*12 + m*4 + co_l)."""
    shifts = [np.asarray([REAL_PAD[int(ix) % NK] for ix in s], np.int32)
              for s in shift_idxs]
    Wm = np.zeros((CH_PER_BLK, N_BLOCKS, NK, N_SETS * CO_PER_BLK), np.float32)
    for i in range(N_BLOCKS):
        for p in range(CH_PER_BLK):
            g, r = divmod(p, 28)
            co_l, k = divmod(r, NK)
            c = g * (C_OUT * NK) + (CO_PER_BLK * i + co_l) * NK + k
            for m in range(N_SETS):
                v = REAL_PAD.index(int(shifts[m][c]))
                Wm[p, i, v, m * CO_PER_BLK + co_l] = 1.0
    return np.ascontiguousarray(Wm.reshape(CH_PER_BLK, N_BLOCKS * NK * 12))


def _build_bands():
    """[128, 13*128]: 7 main shifted-identity bands + 6 corner bands."""
    bands = np.zeros((13, 128, 128), np.float32)
    for vi, s in enumerate(REAL_PAD):
        for u in range(128):
            t = u + s
            if 0 <= t < 128:
                bands[vi, u, t] = 1.0
    ci = 7
    for vi, s in enumerate(REAL_PAD):
        if s == 0:
            continue
        for u in range(128):
            t = u + s - 128 if s > 0 else u + s + 128
            if 0 <= t < 128:
                bands[ci, u, t] = 1.0
        ci += 1
    return np.ascontiguousarray(np.transpose(bands, (1, 0, 2)).reshape(128, 13 * 128))


def _build_kernel():
    n_chunks = L // CHUNK
    n_t = L // TCH
    tpc = CHUNK // TCH
    PW = 1024  # rolling P window (columns)

    nc = bacc.Bacc(None, target_bir_lowering=False, debug=False)
    x = nc.declare_dram_parameter("x", [C_IN, L], BF16, isOutput=False)
    w = nc.declare_dram_parameter("w", [CH_PER_BLK, N_BLOCKS * 84], BF16, isOutput=False)
    ident = nc.declare_dram_parameter("ident", [128, 128], BF16, isOutput=False)
    bands = nc.declare_dram_parameter("bands", [128, 13 * 128], BF16, isOutput=False)
    outT = nc.declare_dram_parameter("outT", [L, W], F32, isOutput=True)

    with tile.TileContext(nc) as tc, ExitStack() as ctx:
        const = ctx.enter_context(tc.tile_pool(name="const", bufs=1))
        xpool = ctx.enter_context(tc.tile_pool(name="x", bufs=N_BLOCKS + 8))
        ppool = ctx.enter_context(tc.tile_pool(name="P", bufs=1))
        ptpool = ctx.enter_context(tc.tile_pool(name="PT", bufs=6))
        otpool = ctx.enter_context(tc.tile_pool(name="oT", bufs=6))
        psA = ctx.enter_context(tc.tile_pool(name="psA", bufs=4, space=bass.MemorySpace.PSUM))
        psT = ctx.enter_context(tc.tile_pool(name="psT", bufs=2, space=bass.MemorySpace.PSUM))
        psS2 = ctx.enter_context(tc.tile_pool(name="psS2", bufs=2, space=bass.MemorySpace.PSUM))

        w_sb = const.tile([CH_PER_BLK, N_BLOCKS * 84], BF16)
        id_sb = const.tile([128, 128], BF16)
        bd_sb = const.tile([128, 13 * 128], BF16)

        P_sb = [ppool.tile([84, PW], BF16, name=f"Psb{i}", tag=f"P{i}")
                for i in range(N_BLOCKS)]
        n_pairs = n_t // 2
        PTP = [None] * n_pairs  # pair tile cols: v*384 + m*128 + h*64 + co

        def emit_s2_pair(pi):
            """S2 for dst chunks (2pi, 2pi+1). Main bands cover both chunks
            in one [128, 2W] matmul; corners pull cross-chunk slices."""
            ja = 2 * pi

            def hview(tile_ap, vi, h):
                """[p, m, co] view of one (v, h) slice; cols v*384+m*128+h*64+co."""
                return tile_ap[:].rearrange("p (v m h co) -> p v m h co",
                                            v=NK, m=N_SETS, h=2)[:, vi, :, h, :]

            ps = psS2.tile([128, 2 * W], F32)
            ps_r = ps[:].rearrange("p (m h co) -> p m h co", m=N_SETS, h=2)
            ops = []  # (band, moving_ap, out_ap)
            for vi in range(NK):
                ops.append((vi, PTP[pi][:, vi * 2 * W:(vi + 1) * 2 * W], ps[:]))
            for vi in (0, 1, 2):  # s>0: dst h <- src chunk j-1
                if pi > 0:
                    ops.append((CORNER_OF[vi], hview(PTP[pi - 1], vi, 1),
                                ps_r[:, :, 0, :]))
                ops.append((CORNER_OF[vi], hview(PTP[pi], vi, 0), ps_r[:, :, 1, :]))
            for vi in (4, 5, 6):  # s<0: dst h <- src chunk j+1
                ops.append((CORNER_OF[vi], hview(PTP[pi], vi, 1), ps_r[:, :, 0, :]))
                if pi + 1 < n_pairs:
                    ops.append((CORNER_OF[vi], hview(PTP[pi + 1], vi, 0),
                                ps_r[:, :, 1, :]))
            for k, (b, mv, out_ap) in enumerate(ops):
                nc.tensor.matmul(out_ap, bd_sb[:, TCH * b:TCH * (b + 1)], mv,
                                 start=(k == 0), stop=(k == len(ops) - 1))
            ot = otpool.tile([128, 2 * W], F32)
            nc.scalar.copy(ot[:], ps[:])
            ot_r = ot[:].rearrange("p (m h co) -> p m h co", m=N_SETS, h=2)
            for h in range(2):
                nc.scalar.dma_start(outT[TCH * (ja + h):TCH * (ja + h + 1), :],
                                    ot_r[:, :, h, :])

        XW = 2 * CHUNK
        xslabs = []
        for jc in range(n_chunks):
            if jc < 2:
                # narrow first slabs: get the pipeline moving sooner
                xslabs = []
                for i in range(N_BLOCKS):
                    xt = xpool.tile([CH_PER_BLK, CHUNK], BF16, name=f"xn{jc}_{i}",
                                    tag="xn")
                    nc.sync.dma_start(xt[:], x[CH_PER_BLK * i:CH_PER_BLK * (i + 1),
                                               CHUNK * jc:CHUNK * (jc + 1)])
                    xslabs.append(xt)
                    if jc == 0 and i == 0:
                        nc.sync.dma_start(w_sb[:], w[:])
                    if jc == 0 and i == 2:
                        nc.sync.dma_start(id_sb[:], ident[:])
                    if jc == 0 and i == 4:
                        nc.sync.dma_start(bd_sb[:], bands[:])
                xts = [xs[:] for xs in xslabs]
            else:
                if jc % 2 == 0:
                    xslabs = []
                    for i in range(N_BLOCKS):
                        xt = xpool.tile([CH_PER_BLK, XW], BF16)
                        nc.sync.dma_start(xt[:], x[CH_PER_BLK * i:CH_PER_BLK * (i + 1),
                                                   CHUNK * jc:CHUNK * jc + XW])
                        xslabs.append(xt)
                xts = [xs[:, (jc % 2) * CHUNK:(jc % 2 + 1) * CHUNK] for xs in xslabs]
            for i in range(N_BLOCKS):
                ps = psA.tile([84, CHUNK], F32)
                nc.tensor.matmul(ps[:], w_sb[:, 84 * i:84 * (i + 1)], xts[i],
                                 start=True, stop=True)
                pc = (CHUNK * jc) % PW
                if i % 2 == 0:
                    nc.vector.tensor_copy(P_sb[i][:, pc:pc + CHUNK], ps[:])
                else:
                    nc.scalar.copy(P_sb[i][:, pc:pc + CHUNK], ps[:])
            for jt in range(tpc * jc, tpc * (jc + 1)):
                pi, hh = divmod(jt, 2)
                if hh == 0:
                    PTP[pi] = ptpool.tile([128, NK * 2 * W], BF16, name=f"ptp{pi}", tag="ptp")
                pt = PTP[pi]
                for h0 in range(0, N_BLOCKS, 8):
                    hblks = list(range(h0, min(h0 + 8, N_BLOCKS)))
                    pst = psT.tile([128, 84 * 8], BF16)
                    for q, i in enumerate(hblks):
                        tcol = (TCH * jt) % PW
                        nc.tensor.transpose(pst[:, 84 * q:84 * (q + 1)],
                                            P_sb[i][:, tcol:tcol + TCH],
                                            id_sb[:84, :84])
                    nq = len(hblks)
                    csrc = pst[:, 0:84 * nq].rearrange(
                        "p (q v m c) -> p q v m c", q=nq, v=NK, m=N_SETS)
                    cdst = pt[:].rearrange(
                        "p (v m h co) -> p v m h co", v=NK, m=N_SETS, h=2)[
                        :, :, :, hh,
                        CO_PER_BLK * h0:CO_PER_BLK * (h0 + nq)].rearrange(
                        "p v m (q c) -> p q v m c", q=nq)
                    if h0 == 0:
                        nc.scalar.copy(cdst, csrc)
                    else:
                        nc.vector.tensor_copy(cdst, csrc)
                if jt >= 2 and jt % 2 == 0:
                    emit_s2_pair(jt // 2 - 1)
        emit_s2_pair(n_pairs - 1)

    nc.compile()
    return nc


def kernel(x, shift_idx_1, shift_idx_2, shift_idx_3):
    global _compiled
    x = np.asarray(x)
    sidx = [np.asarray(shift_idx_1), np.asarray(shift_idx_2), np.asarray(shift_idx_3)]

    if _compiled is None:
        _compiled = _build_kernel()
    nc = _compiled

    Wm = _build_weights(sidx).astype(ml_dtypes.bfloat16)
    bands = _build_bands().astype(ml_dtypes.bfloat16)
    ident = np.eye(128, dtype=np.float32).astype(ml_dtypes.bfloat16)
    ridx = _permute_rows()
    in_maps = []
    for b in range(B):
        xb = np.ascontiguousarray(np.asarray(x[b], np.float32)[ridx]).astype(ml_dtypes.bfloat16)
        in_maps.append({"x": xb, "w": Wm, "ident": ident, "bands": bands})

    res = run_bass_kernel_spmd(nc, in_maps, core_ids=list(range(8)), trace=False)

    outs = []
    for m in range(N_SETS):
        outs.append(np.stack(
            [np.asarray(res.results[b]["outT"][:, m * NCO:(m + 1) * NCO].T,
                        dtype=np.float32) for b in range(B)]))
    return tuple(outs)


# revision 8
# speedup vs baseline: 1.2080x; 1.0631x over previous
"""AddShift segment-reduce kernel for Trainium2, SPMD across 8 NeuronCores.

Problem (hardcoded): x [8, 1792, 4096] f32; channel c = g*448 + co*7 + k
(g<4, co<64, k<7). For each of 3 index sets m, channel c is shifted along
the last axis by s_m(c) = REAL_PAD[shift_idx_m[c] % 7] in {+-15,+-10,+-5,0}
(zero fill) and channels sharing co are summed over (g, k):
  out_m[b, co, t] = sum_{g,k} x[b, c, t - s_m(c)] * in_range
Returns (x1, x2, x3), each [8, 64, 4096] f32.

Sharding: data-parallel over batch — core b computes batch b.

Device algorithm (3 TensorE stages, all bf16 operands, f32 accumulation):
  S1  per channel-block (4 co slots = 112 channels) and 512-col chunk:
      one matmul with a 0/1 selector stationary [112, 84] reduces channels
      into 84 bucket rows (v shift-class x m set x co_l) in PSUM;
      PSUM is drained to SBUF as bf16 (P).
  T   PE-transposes P in 128-col chunks into PT [t, bucket], reordering
      buckets v-major: col = v*192 + m*64 + co.
  S2  banded matmuls: for dst t-chunk j, out[t', (m,co)] += B.T @ PT_v
      where B is a shifted-identity band [128, 128]; corner bands pull the
      <=15-row contributions that cross chunk boundaries from PT[j-1]/PT[j+1].
      Boundary zero-fill falls out of omitted corners / zero band columns.
Output is written transposed (outT [4096, 192]); the host restores layout.

x is shipped as bf16 (host cast) — halves HBM traffic; selector/band
weights are exact 0/1 in bf16. Overall rel. error ~3e-3.
"""
from contextlib import ExitStack

import numpy as np
import ml_dtypes

import concourse.bass as bass
import concourse.mybir as mybir
import concourse.tile as tile
from concourse import bacc
from concourse.bass_utils import run_bass_kernel_spmd

# Problem constants
SMALL_K = 5
C_OUT, GROUP_IN = 64, 4
NK = 7
C_IN = 1792
B, L = 8, 4096
REAL_PAD = [int(((NK - 1) / 2.0 - i) * SMALL_K) for i in range(NK)]  # [15,10,5,0,-5,-10,-15]
N_SETS = 3
CO_PER_BLK = 4
CH_PER_BLK = CO_PER_BLK * GROUP_IN * NK  # 112
N_BLOCKS = C_OUT // CO_PER_BLK           # 16
NCO = C_OUT
W = N_SETS * NCO                          # 192
TCH = 128
CHUNK = 512

F32 = mybir.dt.float32
BF16 = mybir.dt.bfloat16

CORNER_OF = {0: 7, 1: 8, 2: 9, 4: 10, 5: 11, 6: 12}
V_OF_BAND = {0: 0, 1: 1, 2: 2, 3: 3, 4: 4, 5: 5, 6: 6,
             7: 0, 8: 1, 9: 2, 10: 4, 11: 5, 12: 6}

_compiled = None


def _permute_rows():
    idx = np.empty(CH_PER_BLK * N_BLOCKS, np.int64)
    for i in range(N_BLOCKS):
        for p in range(CH_PER_BLK):
            g, r = divmod(p, 28)
            co_l, k = divmod(r, NK)
            idx[i * CH_PER_BLK + p] = g * (C_OUT * NK) + (CO_PER_BLK * i + co_l) * NK + k
    return idx


def _build_weights(shift_idxs):
    """S1 selector [112, N_BLOCKS*84]; block i cols (v*12 + m*4 + co_l)."""
    shifts = [np.asarray([REAL_PAD[int(ix) % NK] for ix in s], np.int32)
              for s in shift_idxs]
    Wm = np.zeros((CH_PER_BLK, N_BLOCKS, NK, N_SETS * CO_PER_BLK), np.float32)
    for i in range(N_BLOCKS):
        for p in range(CH_PER_BLK):
            g, r = divmod(p, 28)
            co_l, k = divmod(r, NK)
            c = g * (C_OUT * NK) + (CO_PER_BLK * i + co_l) * NK + k
            for m in range(N_SETS):
                v = REAL_PAD.index(int(shifts[m][c]))
                Wm[p, i, v, m * CO_PER_BLK + co_l] = 1.0
    return np.ascontiguousarray(Wm.reshape(CH_PER_BLK, N_BLOCKS * NK * 12))


def _build_bands():
    """[128, 13*128]: 7 main shifted-identity bands + 6 corner bands."""
    bands = np.zeros((13, 128, 128), np.float32)
    for vi, s in enumerate(REAL_PAD):
        for u in range(128):
            t = u + s
            if 0 <= t < 128:
                bands[vi, u, t] = 1.0
    ci = 7
    for vi, s in enumerate(REAL_PAD):
        if s == 0:
            continue
        for u in range(128):
            t = u + s - 128 if s > 0 else u + s + 128
            if 0 <= t < 128:
                bands[ci, u, t] = 1.0
        ci += 1
    return np.ascontiguousarray(np.transpose(bands, (1, 0, 2)).reshape(128, 13 * 128))


def _build_kernel():
    n_chunks = L // CHUNK
    n_t = L // TCH
    tpc = CHUNK // TCH
    PW = 1024  # rolling P window (columns)

    nc = bacc.Bacc(None, target_bir_lowering=False, debug=False)
    x = nc.declare_dram_parameter("x", [C_IN, L], BF16, isOutput=False)
    w = nc.declare_dram_parameter("w", [CH_PER_BLK, N_BLOCKS * 84], BF16, isOutput=False)
    ident = nc.declare_dram_parameter("ident", [128, 128], BF16, isOutput=False)
    bands = nc.declare_dram_parameter("bands", [128, 13 * 128], BF16, isOutput=False)
    outT = nc.declare_dram_parameter("outT", [L, W], F32, isOutput=True)

    with tile.TileContext(nc) as tc, ExitStack() as ctx:
        const = ctx.enter_context(tc.tile_pool(name="const", bufs=1))
        xpool = ctx.enter_context(tc.tile_pool(name="x", bufs=N_BLOCKS + 8))
        ppool = ctx.enter_context(tc.tile_pool(name="P", bufs=1))
        ptpool = ctx.enter_context(tc.tile_pool(name="PT", bufs=6))
        otpool = ctx.enter_context(tc.tile_pool(name="oT", bufs=6))
        psA = ctx.enter_context(tc.tile_pool(name="psA", bufs=4, space=bass.MemorySpace.PSUM))
        psT = ctx.enter_context(tc.tile_pool(name="psT", bufs=2, space=bass.MemorySpace.PSUM))
        psS2 = ctx.enter_context(tc.tile_pool(name="psS2", bufs=2, space=bass.MemorySpace.PSUM))

        w_sb = const.tile([CH_PER_BLK, N_BLOCKS * 84], BF16)
        id_sb = const.tile([128, 128], BF16)
        bd_sb = const.tile([128, 13 * 128], BF16)

        P_sb = [ppool.tile([84, PW], BF16, name=f"Psb{i}", tag=f"P{i}")
                for i in range(N_BLOCKS)]
        n_pairs = n_t // 2
        PTP = [None] * n_pairs  # pair tile cols: v*384 + m*128 + h*64 + co

        def emit_s2_pair(pi):
            """S2 for dst chunks (2pi, 2pi+1). Main bands cover both chunks
            in one [128, 2W] matmul; corners pull cross-chunk slices."""
            ja = 2 * pi

            def hview(tile_ap, vi, h):
                """[p, m, co] view of one (v, h) slice; cols v*384+m*128+h*64+co."""
                return tile_ap[:].rearrange("p (v m h co) -> p v m h co",
                                            v=NK, m=N_SETS, h=2)[:, vi, :, h, :]

            ps = psS2.tile([128, 2 * W], F32)
            ps_r = ps[:].rearrange("p (m h co) -> p m h co", m=N_SETS, h=2)
            ops = []  # (band, moving_ap, out_ap)
            for vi in range(NK):
                ops.append((vi, PTP[pi][:, vi * 2 * W:(vi + 1) * 2 * W], ps[:]))
            for vi in (0, 1, 2):  # s>0: dst h <- src chunk j-1
                if pi > 0:
                    ops.append((CORNER_OF[vi], hview(PTP[pi - 1], vi, 1),
                                ps_r[:, :, 0, :]))
                ops.append((CORNER_OF[vi], hview(PTP[pi], vi, 0), ps_r[:, :, 1, :]))
            for vi in (4, 5, 6):  # s<0: dst h <- src chunk j+1
                ops.append((CORNER_OF[vi], hview(PTP[pi], vi, 1), ps_r[:, :, 0, :]))
                if pi + 1 < n_pairs:
                    ops.append((CORNER_OF[vi], hview(PTP[pi + 1], vi, 0),
                                ps_r[:, :, 1, :]))
            for k, (b, mv, out_ap) in enumerate(ops):
                nc.tensor.matmul(out_ap, bd_sb[:, TCH * b:TCH * (b + 1)], mv,
                                 start=(k == 0), stop=(k == len(ops) - 1))
            ot = otpool.tile([128, 2 * W], F32)
            nc.scalar.copy(ot[:], ps[:])
            ot_r = ot[:].rearrange("p (m h co) -> p m h co", m=N_SETS, h=2)
            for h in range(2):
                nc.scalar.dma_start(outT[TCH * (ja + h):TCH * (ja + h + 1), :],
                                    ot_r[:, :, h, :])

        XW = 2 * CHUNK
        xslabs = []
        for jc in range(n_chunks):
            if jc < 2:
                # narrow first slabs: get the pipeline moving sooner
                xslabs = []
                for i in range(N_BLOCKS):
                    xt = xpool.tile([CH_PER_BLK, CHUNK], BF16, name=f"xn{jc}_{i}",
                                    tag="xn")
                    nc.sync.dma_start(xt[:], x[CH_PER_BLK * i:CH_PER_BLK * (i + 1),
                                               CHUNK * jc:CHUNK * (jc + 1)])
                    xslabs.append(xt)
                    if jc == 0 and i == 0:
                        nc.sync.dma_start(w_sb[:], w[:])
                    if jc == 0 and i == 2:
                        nc.sync.dma_start(id_sb[:], ident[:])
                    if jc == 0 and i == 4:
                        nc.sync.dma_start(bd_sb[:], bands[:])
                xts = [xs[:] for xs in xslabs]
            else:
                if jc % 2 == 0:
                    xslabs = []
                    for i in range(N_BLOCKS):
                        xt = xpool.tile([CH_PER_BLK, XW], BF16)
                        nc.sync.dma_start(xt[:], x[CH_PER_BLK * i:CH_PER_BLK * (i + 1),
                                                   CHUNK * jc:CHUNK * jc + XW])
                        xslabs.append(xt)
                xts = [xs[:, (jc % 2) * CHUNK:(jc % 2 + 1) * CHUNK] for xs in xslabs]
            for i in range(N_BLOCKS):
                if i == 0 and 0 <= 2 * jc - 3 < n_pairs:
                    emit_s2_pair(2 * jc - 3)
                if i == 8 and 0 <= 2 * jc - 2 < n_pairs:
                    emit_s2_pair(2 * jc - 2)
                ps = psA.tile([84, CHUNK], F32)
                nc.tensor.matmul(ps[:], w_sb[:, 84 * i:84 * (i + 1)], xts[i],
                                 start=True, stop=True)
                pc = (CHUNK * jc) % PW
                if i % 2 == 0:
                    nc.vector.tensor_copy(P_sb[i][:, pc:pc + CHUNK], ps[:])
                else:
                    nc.scalar.copy(P_sb[i][:, pc:pc + CHUNK], ps[:])
            for jt in range(tpc * jc, tpc * (jc + 1)):
                pi, hh = divmod(jt, 2)
                if hh == 0:
                    PTP[pi] = ptpool.tile([128, NK * 2 * W], BF16, name=f"ptp{pi}", tag="ptp")
                pt = PTP[pi]
                for h0 in range(0, N_BLOCKS, 8):
                    hblks = list(range(h0, min(h0 + 8, N_BLOCKS)))
                    pst = psT.tile([128, 84 * 8], BF16)
                    for q, i in enumerate(hblks):
                        tcol = (TCH * jt) % PW
                        nc.tensor.transpose(pst[:, 84 * q:84 * (q + 1)],
                                            P_sb[i][:, tcol:tcol + TCH],
                                            id_sb[:84, :84])
                    nq = len(hblks)
                    csrc = pst[:, 0:84 * nq].rearrange(
                        "p (q v m c) -> p q v m c", q=nq, v=NK, m=N_SETS)
                    cdst = pt[:].rearrange(
                        "p (v m h co) -> p v m h co", v=NK, m=N_SETS, h=2)[
                        :, :, :, hh,
                        CO_PER_BLK * h0:CO_PER_BLK * (h0 + nq)].rearrange(
                        "p v m (q c) -> p q v m c", q=nq)
                    if h0 == 0:
                        nc.scalar.copy(cdst, csrc)
                    else:
                        nc.vector.tensor_copy(cdst, csrc)
        for p_left in range(max(0, 2 * n_chunks - 3), n_pairs):
            emit_s2_pair(p_left)

    nc.compile()
    return nc


def kernel(x, shift_idx_1, shift_idx_2, shift_idx_3):
    global _compiled
    x = np.asarray(x)
    sidx = [np.asarray(shift_idx_1), np.asarray(shift_idx_2), np.asarray(shift_idx_3)]

    if _compiled is None:
        _compiled = _build_kernel()
    nc = _compiled

    Wm = _build_weights(sidx).astype(ml_dtypes.bfloat16)
    bands = _build_bands().astype(ml_dtypes.bfloat16)
    ident = np.eye(128, dtype=np.float32).astype(ml_dtypes.bfloat16)
    ridx = _permute_rows()
    in_maps = []
    for b in range(B):
        xb = np.ascontiguousarray(np.asarray(x[b], np.float32)[ridx]).astype(ml_dtypes.bfloat16)
        in_maps.append({"x": xb, "w": Wm, "ident": ident, "bands": bands})

    res = run_bass_kernel_spmd(nc, in_maps, core_ids=list(range(8)), trace=False)

    outs = []
    for m in range(N_SETS):
        outs.append(np.stack(
            [np.asarray(res.results[b]["outT"][:, m * NCO:(m + 1) * NCO].T,
                        dtype=np.float32) for b in range(B)]))
    return tuple(outs)
